# revision 6
# baseline (speedup 1.0000x reference)
"""Trainium2 Bass kernel for nn_CuteInferMLP (fp8-emulated dense MLP).

Sharding: tensor-parallel over the intermediate dim I=14336 across 8 cores,
activations replicated, output reduce-scattered to a data-parallel token
sharding (per the sharding hint).

Host side prepares the fp8-quant-dequantized operands (Xd, W0d, W1d) in
fp16 exactly matching the reference recipe (per-row per-128-chunk e4m3fn).
The device kernel per core:
  - GEMM1 produces D^T = [i_local, m] tiles (lhsT = W0d^T via DMA-transpose
    loads, rhs = Xd^T), with bias0 + exact-erf GELU fused in the ACT epilogue.
  - D is re-quantized on device in the transposed layout (gpsimd partition
    absmax all-reduce + partition broadcast of the scales; TRN fp8e4 with a
    2x-halved scale reproduces the OCP e4m3fn grid, which saturates at 448
    while TRN saturates at 240).
  - GEMM2 accumulates E_partial[m, h] over the local i-chunks with W1d^T
    resident in SBUF; bias1/8 is added via a rank-1 matmul on every core.
  - A per-token-group bf16 ReduceScatter sums partials across cores.
"""

import os

import numpy as np
import ml_dtypes

import concourse.bass as bass
import concourse.mybir as mybir
import concourse.tile as tile
from concourse import bacc
from concourse.bass_isa import ReduceOp
from concourse.bass_utils import run_bass_kernel_spmd

BF16 = mybir.dt.bfloat16
FP32 = mybir.dt.float32
FP16 = mybir.dt.float16
FP8 = mybir.dt.float8e4
AF = mybir.ActivationFunctionType
ALU = mybir.AluOpType

P = 128
CHUNK = 128


def build_program(n_cores, M, H, I_loc, m_group, h_seg=512):
    """Build the SPMD program (identical on all cores)."""
    assert M % m_group == 0 and m_group % P == 0
    assert H % CHUNK == 0 and I_loc % CHUNK == 0
    n_groups = M // m_group
    mt_per_g = m_group // P
    KH = H // CHUNK          # k-chunks of GEMM1 (contraction H)
    KI = I_loc // CHUNK      # i-chunks (contraction of GEMM2)
    assert H % h_seg == 0
    n_hseg = H // h_seg
    CI = I_loc // CHUNK
    rs_rows = m_group // n_cores
    assert m_group % n_cores == 0

    # i-tile grouping for GEMM1 psum (<=4 banks in flight)
    IG = []
    it0 = 0
    while it0 < KI:
        IG.append(list(range(it0, min(it0 + 4, KI))))
        it0 += 4

    nc = bacc.Bacc(
        "TRN2", target_bir_lowering=False, debug=False, num_devices=n_cores
    )

    xdn = nc.dram_tensor("Xd", (M, H), FP16, kind="ExternalInput").ap()
    w0dn = nc.dram_tensor("W0d", (I_loc, H), FP16, kind="ExternalInput").ap()
    b0s = nc.dram_tensor("b0s", (I_loc,), BF16, kind="ExternalInput").ap()
    w1dn = nc.dram_tensor("W1d", (H, I_loc), FP16, kind="ExternalInput").ap()
    b1e = nc.dram_tensor("b1e", (H,), BF16, kind="ExternalInput").ap()
    eout = nc.dram_tensor("Eout", (M // n_cores, H), BF16, kind="ExternalOutput").ap()

    with tile.TileContext(nc) as tc:
        with (
            tc.tile_pool(name="dram", bufs=1, space="DRAM") as dram,
            tc.tile_pool(name="consts", bufs=1) as consts,
            tc.tile_pool(name="w1res", bufs=1) as w1res,
            tc.tile_pool(name="xdt", bufs=1) as xdtp,
            tc.tile_pool(name="w0t", bufs=4) as w0tp,
            tc.tile_pool(name="ddqt", bufs=1) as ddqtp,
            tc.tile_pool(name="dwork", bufs=3) as dwork,
            tc.tile_pool(name="dsc", bufs=1) as dscp,
            tc.tile_pool(name="esb", bufs=3) as esbp,
            tc.tile_pool(name="ps_g1", bufs=5, space="PSUM") as ps_g1,
            tc.tile_pool(name="ps_g2", bufs=2, space="PSUM") as ps_g2,
        ):
            epart = dram.tile([M, H], BF16)
            rsout = dram.tile([M // n_cores, H], BF16)

            # constants
            ones_t = consts.tile([1, P], BF16)
            nc.any.memset(ones_t[:], 1.0)
            b1_sb = consts.tile([1, H], BF16)
            nc.sync.dma_start(b1_sb[:], b1e[None, :])
            b0_sb = consts.tile([P, CI], BF16)
            nc.sync.dma_start(b0_sb[:], b0s.rearrange("(t p) -> p t", p=P))
            b0_f32 = consts.tile([P, CI], FP32)
            nc.vector.tensor_copy(b0_f32[:], b0_sb[:])

            # W1d^T resident: [128 i, KI, H]
            w1dt = w1res.tile([P, KI, H], FP16)
            for k in range(KI):
                nc.sync.dma_start_transpose(
                    w1dt[:, k, :], w1dn[:, k * P : (k + 1) * P]
                )

            for g in range(n_groups):
                r0 = g * m_group
                # Xd^T for the group: [128 h, KH, m_group]
                xdt = xdtp.tile([P, KH, m_group], FP16)
                for k in range(KH):
                    nc.sync.dma_start_transpose(
                        xdt[:, k, :], xdn[r0 : r0 + m_group, k * P : (k + 1) * P]
                    )

                ddqt = ddqtp.tile([P, KI, m_group], FP16)

                # ---- GEMM1 + gelu + D-requant ----
                for ig in IG:
                    psums = {}
                    for it in ig:
                        psums[it] = ps_g1.tile(
                            [P, 512], FP32, tag="g1", name="g1"
                        )[:, :m_group]
                    niw = len(ig)
                    for k in range(KH):
                        w0t = w0tp.tile([P, 512], FP16, tag="w0t", name="w0t")[
                            :, : niw * P
                        ]
                        nc.sync.dma_start_transpose(
                            w0t,
                            w0dn[ig[0] * P : ig[0] * P + niw * P,
                                 k * P : (k + 1) * P],
                        )
                        for j, it in enumerate(ig):
                            nc.tensor.matmul(
                                psums[it],
                                w0t[:, j * P : (j + 1) * P],
                                xdt[:, k, :],
                                start=(k == 0),
                                stop=(k == KH - 1),
                            )
                    for it in ig:
                        dt_sb = dwork.tile([P, 512], BF16, tag="dt", name="dt")[
                            :, :m_group
                        ]
                        nc.scalar.activation(
                            dt_sb, psums[it], AF.Gelu,
                            bias=b0_f32[:, it : it + 1],
                        )
                        am = dscp.tile([P, 512], FP32, tag="dam", name="dam")[
                            :, :m_group
                        ]
                        nc.gpsimd.partition_all_reduce(
                            am, dt_sb, P, ReduceOp.absmax
                        )
                        trow = dscp.tile([1, 512], FP32, tag="dt_t", name="dt_t")[
                            :, :m_group
                        ]
                        nc.vector.tensor_scalar(
                            trow, am[0:1, :], 1e-4, None, op0=ALU.max
                        )
                        i2r = dscp.tile([1, 512], FP32, tag="dt_i", name="dt_i")[
                            :, :m_group
                        ]
                        nc.vector.reciprocal(i2r, trow)
                        nc.vector.tensor_scalar(i2r, i2r, 224.0, None, op0=ALU.mult)
                        s2r = dscp.tile([1, 512], FP32, tag="dt_s", name="dt_s")[
                            :, :m_group
                        ]
                        nc.vector.tensor_scalar(
                            s2r, trow, 1.0 / 224.0, None, op0=ALU.mult
                        )
                        i2b = dscp.tile([P, 512], FP32, tag="dt_ib", name="dt_ib")[
                            :, :m_group
                        ]
                        nc.gpsimd.partition_broadcast(i2b, i2r, P)
                        s2b = dscp.tile([P, 512], FP32, tag="dt_sb", name="dt_sb")[
                            :, :m_group
                        ]
                        nc.gpsimd.partition_broadcast(s2b, s2r, P)
                        qd = dwork.tile([P, 512], FP8, tag="dq8", name="dq8")[
                            :, :m_group
                        ]
                        nc.vector.tensor_tensor(qd, dt_sb, i2b, op=ALU.mult)
                        nc.vector.tensor_tensor(
                            ddqt[:, it, :], qd, s2b, op=ALU.mult
                        )

                # ---- GEMM2 ----
                for mt in range(mt_per_g):
                    for hs in range(n_hseg):
                        ps2 = ps_g2.tile([P, h_seg], FP32, tag="g2", name="g2")
                        nc.tensor.matmul(
                            ps2, ones_t[:],
                            b1_sb[:, hs * h_seg : (hs + 1) * h_seg],
                            start=True, stop=False,
                        )
                        for k in range(KI):
                            nc.tensor.matmul(
                                ps2,
                                ddqt[:, k, mt * P : (mt + 1) * P],
                                w1dt[:, k, hs * h_seg : (hs + 1) * h_seg],
                                start=False, stop=(k == KI - 1),
                            )
                        e_sb = esbp.tile([P, h_seg], BF16, tag="esb", name="esb")
                        nc.scalar.copy(e_sb, ps2)
                        nc.sync.dma_start(
                            epart[r0 + mt * P : r0 + (mt + 1) * P,
                                  hs * h_seg : (hs + 1) * h_seg],
                            e_sb,
                        )

                if os.environ.get("KERNEL_NO_RS"):
                    nc.sync.dma_start(
                        rsout[g * rs_rows : (g + 1) * rs_rows, :],
                        epart[r0 : r0 + rs_rows, :],
                    )
                else:
                    nc.gpsimd.collective_compute(
                        "ReduceScatter",
                        ALU.add,
                        replica_groups=[list(range(n_cores))],
                        ins=[epart[r0 : r0 + m_group, :].opt()],
                        outs=[rsout[g * rs_rows : (g + 1) * rs_rows, :].opt()],
                    )
                nc.sync.dma_start(
                    eout[g * rs_rows : (g + 1) * rs_rows, :],
                    rsout[g * rs_rows : (g + 1) * rs_rows, :],
                )

    nc.compile()
    return nc


# ---------------------------------------------------------------------------
# Host-side quantization (exactly the reference recipe) + driver
# ---------------------------------------------------------------------------


def host_qdq_fp16(x_f32):
    """Reference-exact per-row per-128-chunk e4m3fn quant-dequant, fp16 out."""
    M, Kd = x_f32.shape
    C = Kd // CHUNK
    xr = x_f32.reshape(M, C, CHUNK)
    amax = np.abs(xr).max(-1, keepdims=True)
    s = (np.maximum(amax, np.float32(1e-4)) / np.float32(448.0)).astype(np.float32)
    q = (xr / s).astype(ml_dtypes.float8_e4m3fn)
    return (q.astype(np.float32) * s).astype(np.float16).reshape(M, Kd)


_N_CORES = 8
_B, _L, _H, _I = 2, 4096, 4096, 14336
_M = _B * _L
_M_GROUP = 512

_program_cache = {}


def _get_program(*args):
    if args not in _program_cache:
        _program_cache[args] = build_program(*args)
    return _program_cache[args]


def run(X0, W0, bias0, W1, bias1, n_cores, M, H, I, m_group, h_seg=512):
    i_loc = I // n_cores
    nc = _get_program(n_cores, M, H, i_loc, m_group, h_seg)
    # host prep: bf16 cast of X (reference semantics), then quant-dequant
    xb = X0.reshape(M, H).astype(ml_dtypes.bfloat16).astype(np.float32)
    Xd = host_qdq_fp16(xb)
    W0d = host_qdq_fp16(np.ascontiguousarray(W0).astype(np.float32))
    W1d = host_qdq_fp16(np.ascontiguousarray(W1).astype(np.float32))
    b1e = (bias1.astype(np.float32) * (1.0 / n_cores)).astype(ml_dtypes.bfloat16)
    in_maps = []
    for r in range(n_cores):
        in_maps.append(
            {
                "Xd": Xd,
                "W0d": np.ascontiguousarray(W0d[r * i_loc : (r + 1) * i_loc, :]),
                "b0s": np.ascontiguousarray(bias0[r * i_loc : (r + 1) * i_loc]),
                "W1d": np.ascontiguousarray(W1d[:, r * i_loc : (r + 1) * i_loc]),
                "b1e": b1e,
            }
        )
    res = run_bass_kernel_spmd(nc, in_maps, core_ids=list(range(n_cores)))
    rs = m_group // n_cores
    E = np.empty((M, H), dtype=ml_dtypes.bfloat16)
    for r in range(n_cores):
        er = res.results[r]["Eout"]
        for g in range(M // m_group):
            E[g * m_group + r * rs : g * m_group + (r + 1) * rs] = er[
                g * rs : (g + 1) * rs
            ]
    return E, res


def kernel(X0, W0, bias0, W1, bias1):
    E, _ = run(X0, W0, bias0, W1, bias1, _N_CORES, _M, _H, _I, _M_GROUP)
    return E.reshape(_B, _L, _H)
